# revision 3
# baseline (speedup 1.0000x reference)
"""Trainium2 Bass kernel for nn_ConvSelfAttention (B=8, H=W=64, C=64, C8=8).

Sharding: data-parallel over batch; core b computes batch item b entirely
on-chip (one-pass softmax attention, no (N,N) matrix ever hits HBM).

Per-core pipeline:
  x [4096,64] -> PE-transpose -> xT [64,4096]
  convs as matmuls from xT:  f4/g4 [128,4096] (f,g replicated at partition
  bases 0/32/64/96 so K=8 score matmuls can use 4x row-tiling), hT [64,4096]
  maxpool(2x2,s1,SAME) via shifted tensor_max on DVE (f4 and hT)
  hT -> PE-transpose -> h_nt [n,c] layout
  main loop over 32 query tiles (128 rows each):
    e = fT.T @ gT   (PSUM, 4x row-tiled)
    p = exp(e)      (ScalarE, fused row-sum accum_out -> S)
    h' = h * (1/S)  (DVE)
    O[c,m] += h'.T @ p  (PSUM accumulate across all 32 tiles, 2x col-tiled:
                         m<2048 -> psum partitions 0:64, else 64:128)
  epilogue: reshape (C,N)->(H,W,C) via 64 PE 64x64 block transposes,
  conv with (gamma*Ko), residual add with x, DMA out.
"""

import numpy as np

import concourse.bass as bass
import concourse.mybir as mybir
import concourse.tile as tile
from concourse import bacc
from concourse.bass_utils import run_bass_kernel_spmd
from concourse.masks import make_identity

FP32 = mybir.dt.float32
AF = mybir.ActivationFunctionType

B, H, W, C = 8, 64, 64, 64
C8 = 8
N = H * W          # 4096
P = 128
NT = N // P        # 32 query tiles
NCORES = 8

_TRACE = False
_LAST_RESULT = None
_CACHED_NC = None


def _build_nc():
    nc = bacc.Bacc("TRN2", target_bir_lowering=False, debug=False,
                   num_devices=NCORES)

    x_d = nc.dram_tensor("x", [N, C], FP32, kind="ExternalInput")
    kf_d = nc.dram_tensor("kf", [C, C8], FP32, kind="ExternalInput")
    kg_d = nc.dram_tensor("kg", [C, C8], FP32, kind="ExternalInput")
    kh_d = nc.dram_tensor("kh", [C, C], FP32, kind="ExternalInput")
    ko_d = nc.dram_tensor("ko", [C, C], FP32, kind="ExternalInput")
    y_d = nc.dram_tensor("y", [N, C], FP32, kind="ExternalOutput")

    with tile.TileContext(nc) as tc:
        _kernel_body(nc, tc, x_d, kf_d, kg_d, kh_d, ko_d, y_d)
    nc.compile()
    return nc


def _pool2x2_s1_same(nc, src, tmp, pp):
    """maxpool window 2x2 stride 1 SAME over trailing (H, W) of [pp, 64, 64].

    src/tmp are flat [pp, 4096] APs viewed as [pp, i, j]; result written back
    into src (in place, via tmp)."""
    s3 = src.rearrange("p (i j) -> p i j", j=W)
    t3 = tmp.rearrange("p (i j) -> p i j", j=W)
    # horizontal: tmp[i, j] = max(src[i, j], src[i, j+1]) (last col pass-through)
    nc.vector.tensor_max(t3[:, :, 0:W - 1], s3[:, :, 0:W - 1], s3[:, :, 1:W])
    nc.vector.tensor_copy(t3[:, :, W - 1:W], s3[:, :, W - 1:W])
    # vertical: src[i, j] = max(tmp[i, j], tmp[i+1, j])
    nc.vector.tensor_max(s3[:, 0:H - 1, :], t3[:, 0:H - 1, :], t3[:, 1:H, :])
    nc.vector.tensor_copy(s3[:, H - 1:H, :], t3[:, H - 1:H, :])


def _kernel_body(nc, tc, x_d, kf_d, kg_d, kh_d, ko_d, y_d):
    with (
        tc.tile_pool(name="const", bufs=1) as cpool,
        tc.tile_pool(name="persist", bufs=1) as ppool,
    ):
        # ---- constants / weights -------------------------------------------
        ident = cpool.tile([P, P], FP32)
        make_identity(nc, ident[:])

        w_f4 = cpool.tile([C, P], FP32)     # Kf replicated at cols 32g..32g+8
        w_g4 = cpool.tile([C, P], FP32)
        w_h = cpool.tile([C, C], FP32)
        w_o = cpool.tile([C, C], FP32)
        nc.vector.memset(w_f4[:], 0.0)
        nc.vector.memset(w_g4[:], 0.0)
        for g in range(4):
            nc.sync.dma_start(w_f4[:, 32 * g:32 * g + C8], kf_d[:, :])
            nc.sync.dma_start(w_g4[:, 32 * g:32 * g + C8], kg_d[:, :])
        nc.sync.dma_start(w_h[:], kh_d[:, :])
        nc.sync.dma_start(w_o[:], ko_d[:, :])

        # ---- persistent activations ----------------------------------------
        xin = ppool.tile([P, NT, C], FP32)      # x, natural [n, c] layout
        f4 = ppool.tile([P, N], FP32)           # pooled fT, replicated 4x
        g4 = ppool.tile([P, N], FP32)           # gT, replicated 4x
        h_nt = ppool.tile([P, NT, C], FP32)     # pooled h in [n, c] layout
        o_rT = ppool.tile([C, N], FP32)         # reshaped-O transposed
        o_sb = ppool.tile([P, 2 * N // 4], FP32)  # O copied out of PSUM

        for t in range(NT):
            nc.sync.dma_start(xin[:, t, :], x_d[P * t:P * t + P, :])

        # ---- prepass: xT, convs, pooling, h transposes ---------------------
        with (
            tc.tile_pool(name="pre", bufs=1) as pre,
            tc.tile_pool(name="pre_ps", bufs=4, space="PSUM") as pre_ps,
        ):
            xT = pre.tile([C, N], FP32)
            hT = pre.tile([C, N], FP32)
            tmp = pre.tile([P, N], FP32)

            # xT[c, n] via PE transposes (4 tiles per PSUM bank)
            for gidx in range(NT // 4):
                tp = pre_ps.tile([C, 4 * P], FP32, tag="pp")
                for j in range(4):
                    t = 4 * gidx + j
                    nc.tensor.transpose(tp[:, P * j:P * j + P], xin[:, t, :],
                                        ident[:])
                nc.vector.tensor_copy(xT[:, 4 * P * gidx:4 * P * (gidx + 1)],
                                      tp[:])

            # 1x1 convs as matmuls from xT
            for blk in range(8):
                rhs = xT[:, 512 * blk:512 * blk + 512]
                psf = pre_ps.tile([P, 512], FP32, tag="pp")
                nc.tensor.matmul(psf[:], w_f4[:], rhs, start=True, stop=True)
                nc.vector.tensor_copy(f4[:, 512 * blk:512 * blk + 512], psf[:])
                psg = pre_ps.tile([P, 512], FP32, tag="pp")
                nc.tensor.matmul(psg[:], w_g4[:], rhs, start=True, stop=True)
                nc.vector.tensor_copy(g4[:, 512 * blk:512 * blk + 512], psg[:])
                psh = pre_ps.tile([C, 512], FP32, tag="pp")
                nc.tensor.matmul(psh[:], w_h[:], rhs, start=True, stop=True)
                nc.vector.tensor_copy(hT[:, 512 * blk:512 * blk + 512], psh[:])

            # maxpool f (all 128 partitions: the 4 replicas pool identically)
            _pool2x2_s1_same(nc, f4[:], tmp[:], P)
            # maxpool h (64 partitions)
            _pool2x2_s1_same(nc, hT[:], tmp[0:C, :], C)

            # h_nt[n, c] via PE transposes (8 tiles per PSUM bank)
            for gidx in range(NT // 8):
                tp = pre_ps.tile([P, 8 * C], FP32, tag="pp")
                for j in range(8):
                    t = 8 * gidx + j
                    nc.tensor.transpose(tp[:, C * j:C * j + C],
                                        hT[:, P * t:P * t + P],
                                        ident[0:C, 0:C])
                nc.vector.tensor_copy(
                    h_nt[:, 8 * gidx:8 * (gidx + 1), :].rearrange(
                        "p t c -> p (t c)"),
                    tp[:])

        # ---- main attention loop -------------------------------------------
        with (
            tc.tile_pool(name="o_ps", bufs=1, space="PSUM") as o_ps_pool,
            tc.tile_pool(name="e_ps", bufs=2, space="PSUM") as e_ps_pool,
            tc.tile_pool(name="p_sb", bufs=2) as p_pool,
            tc.tile_pool(name="s_sb", bufs=2) as s_pool,
        ):
            o_ps = o_ps_pool.tile([P, 2048], FP32)

            for t in range(NT):
                p_t = p_pool.tile([P, N], FP32, tag="p")
                s_parts = s_pool.tile([P, 4], FP32, tag="sp")
                for cch in range(4):
                    e_ps = e_ps_pool.tile([P, 1024], FP32, tag="e")
                    for j in range(2):
                        b = 2 * cch + j
                        grp = b % 4
                        nc.tensor.matmul(
                            e_ps[:, 512 * j:512 * j + 512],
                            f4[32 * grp:32 * grp + C8, P * t:P * t + P],
                            g4[32 * grp:32 * grp + C8, 512 * b:512 * b + 512],
                            start=True, stop=True,
                            tile_position=(32 * grp, 0))
                    nc.scalar.activation(
                        p_t[:, 1024 * cch:1024 * cch + 1024], e_ps[:],
                        AF.Exp, accum_out=s_parts[:, cch:cch + 1])

                s_sum = s_pool.tile([P, 1], tag="ss", dtype=FP32)
                r_t = s_pool.tile([P, 1], tag="rr", dtype=FP32)
                nc.vector.reduce_sum(s_sum[:], s_parts[:],
                                     axis=mybir.AxisListType.X)
                nc.vector.reciprocal(r_t[:], s_sum[:])
                hp_t = s_pool.tile([P, C], tag="hp", dtype=FP32)
                nc.vector.tensor_scalar_mul(hp_t[:], h_nt[:, t, :], r_t[:])

                for b in range(8):
                    obase = 64 * (b // 4)
                    col = 512 * (b % 4)
                    nc.tensor.matmul(
                        o_ps[obase:obase + 64, col:col + 512],
                        hp_t[:],
                        p_t[:, 512 * b:512 * b + 512],
                        start=(t == 0), stop=(t == NT - 1))

            nc.vector.tensor_copy(o_sb[:], o_ps[:])

        # ---- epilogue: reshape transposes, final conv, residual, store -----
        with (
            tc.tile_pool(name="ep_ps", bufs=2, space="PSUM") as ep_ps,
            tc.tile_pool(name="y_sb", bufs=2) as y_pool,
        ):
            # o_rT[k, 64a+w] = O[a, 64w + k];  O[a, m]: partitions 0:64 hold
            # m<2048, partitions 64:128 hold m>=2048 (both at col m%2048).
            o_rT3 = o_rT.rearrange("k (a w) -> k a w", w=W)
            for gidx in range(8):
                tp = ep_ps.tile([C, 8 * C], FP32, tag="ort")
                for j in range(8):
                    w_ = 8 * gidx + j
                    if w_ < 32:
                        src = o_sb[0:64, 64 * w_:64 * w_ + 64]
                        idn = ident[0:64, 0:64]
                    else:
                        src = o_sb[64:128, 64 * w_ - 2048:64 * w_ - 2048 + 64]
                        idn = ident[64:128, 64:128]
                    nc.tensor.transpose(tp[:, C * j:C * j + C], src, idn)
                nc.vector.tensor_copy(
                    o_rT3[:, :, 8 * gidx:8 * (gidx + 1)],
                    tp.rearrange("k (j a) -> k a j", a=C))

            xin_f = xin.rearrange("p t c -> p (t c)")
            for gidx in range(4):
                fps = ep_ps.tile([P, 512], FP32, tag="fin")
                for j in range(8):
                    t = 8 * gidx + j
                    nc.tensor.matmul(fps[:, C * j:C * j + C],
                                     o_rT[:, P * t:P * t + P], w_o[:],
                                     start=True, stop=True)
                y_sb = y_pool.tile([P, 512], FP32, tag="y")
                nc.vector.tensor_add(y_sb[:], fps[:],
                                     xin_f[:, 512 * gidx:512 * gidx + 512])
                for j in range(8):
                    t = 8 * gidx + j
                    nc.sync.dma_start(y_d[P * t:P * t + P, :],
                                      y_sb[:, C * j:C * j + C])


def _get_nc():
    global _CACHED_NC
    if _CACHED_NC is None:
        _CACHED_NC = _build_nc()
    return _CACHED_NC


def kernel(**inputs):
    global _LAST_RESULT
    x = np.ascontiguousarray(np.asarray(inputs["inputs"], dtype=np.float32))
    kf = np.ascontiguousarray(
        np.asarray(inputs["kernel_f"], dtype=np.float32).reshape(C, C8))
    kg = np.ascontiguousarray(
        np.asarray(inputs["kernel_g"], dtype=np.float32).reshape(C, C8))
    kh = np.ascontiguousarray(
        np.asarray(inputs["kernel_h"], dtype=np.float32).reshape(C, C))
    gamma = float(np.asarray(inputs["gamma"]).reshape(()))
    ko = np.ascontiguousarray(
        np.asarray(inputs["kernel_o"], dtype=np.float32).reshape(C, C) * gamma)

    nc = _get_nc()
    in_maps = []
    for b in range(NCORES):
        in_maps.append({
            "x": np.ascontiguousarray(x[b].reshape(N, C)),
            "kf": kf, "kg": kg, "kh": kh, "ko": ko,
        })
    res = run_bass_kernel_spmd(nc, in_maps, core_ids=list(range(NCORES)),
                               trace=_TRACE)
    _LAST_RESULT = res
    out = np.stack(
        [res.results[b]["y"].reshape(H, W, C) for b in range(NCORES)], axis=0)
    return out


# revision 4
# speedup vs baseline: 2.0416x; 2.0416x over previous
"""Trainium2 Bass kernel for nn_ConvSelfAttention (B=8, H=W=64, C=64, C8=8).

Sharding: data-parallel over batch; core b computes batch item b entirely
on-chip (one-pass softmax attention, no (N,N) matrix ever hits HBM).

Per-core pipeline:
  x [4096,64] -> PE-transpose -> xT [64,4096]
  convs as matmuls from xT:  f4/g4 [128,4096] (f,g replicated at partition
  bases 0/32/64/96 so K=8 score matmuls can use 4x row-tiling), hT [64,4096]
  maxpool(2x2,s1,SAME) via shifted tensor_max on DVE (f4 and hT)
  hT -> PE-transpose -> h_nt [n,c] layout
  main loop over 32 query tiles (128 rows each):
    e = fT.T @ gT   (PSUM, 4x row-tiled)
    p = exp(e)      (ScalarE, fused row-sum accum_out -> S)
    h' = h * (1/S)  (DVE)
    O[c,m] += h'.T @ p  (PSUM accumulate across all 32 tiles, 2x col-tiled:
                         m<2048 -> psum partitions 0:64, else 64:128)
  epilogue: reshape (C,N)->(H,W,C) via 64 PE 64x64 block transposes,
  conv with (gamma*Ko), residual add with x, DMA out.
"""

import numpy as np

import concourse.bass as bass
import concourse.mybir as mybir
import concourse.tile as tile
from concourse import bacc
from concourse.bass_utils import run_bass_kernel_spmd
from concourse.masks import make_identity

FP32 = mybir.dt.float32
BF16 = mybir.dt.bfloat16
AF = mybir.ActivationFunctionType

B, H, W, C = 8, 64, 64, 64
C8 = 8
N = H * W          # 4096
P = 128
NT = N // P        # 32 query tiles
NCORES = 8

_TRACE = False
_LAST_RESULT = None
_CACHED_NC = None


def _build_nc():
    nc = bacc.Bacc("TRN2", target_bir_lowering=False, debug=False,
                   num_devices=NCORES)

    x_d = nc.dram_tensor("x", [N, C], FP32, kind="ExternalInput")
    kf_d = nc.dram_tensor("kf", [C, C8], FP32, kind="ExternalInput")
    kg_d = nc.dram_tensor("kg", [C, C8], FP32, kind="ExternalInput")
    kh_d = nc.dram_tensor("kh", [C, C], FP32, kind="ExternalInput")
    ko_d = nc.dram_tensor("ko", [C, C], FP32, kind="ExternalInput")
    y_d = nc.dram_tensor("y", [N, C], FP32, kind="ExternalOutput")

    with tile.TileContext(nc) as tc:
        _kernel_body(nc, tc, x_d, kf_d, kg_d, kh_d, ko_d, y_d)
    nc.compile()
    return nc


def _pool2x2_s1_same(nc, src, tmp, pp):
    """maxpool window 2x2 stride 1 SAME over trailing (H, W) of [pp, 64, 64].

    src/tmp are flat [pp, 4096] APs viewed as [pp, i, j]; result written back
    into src (in place, via tmp)."""
    s3 = src.rearrange("p (i j) -> p i j", j=W)
    t3 = tmp.rearrange("p (i j) -> p i j", j=W)
    # horizontal: tmp[i, j] = max(src[i, j], src[i, j+1]) (last col pass-through)
    nc.vector.tensor_max(t3[:, :, 0:W - 1], s3[:, :, 0:W - 1], s3[:, :, 1:W])
    nc.vector.tensor_copy(t3[:, :, W - 1:W], s3[:, :, W - 1:W])
    # vertical: src[i, j] = max(tmp[i, j], tmp[i+1, j])
    nc.vector.tensor_max(s3[:, 0:H - 1, :], t3[:, 0:H - 1, :], t3[:, 1:H, :])
    nc.vector.tensor_copy(s3[:, H - 1:H, :], t3[:, H - 1:H, :])


def _kernel_body(nc, tc, x_d, kf_d, kg_d, kh_d, ko_d, y_d):
    with (
        tc.tile_pool(name="const", bufs=1) as cpool,
        tc.tile_pool(name="persist", bufs=1) as ppool,
    ):
        # ---- constants / weights -------------------------------------------
        ident = cpool.tile([P, P], FP32)
        make_identity(nc, ident[:])
        identb = cpool.tile([P, P], BF16)
        make_identity(nc, identb[:])

        w_f4 = cpool.tile([C, P], FP32)     # Kf replicated at cols 32g..32g+8
        w_g4 = cpool.tile([C, P], FP32)
        w_h = cpool.tile([C, C], FP32)
        w_o = cpool.tile([C, C], FP32)
        nc.vector.memset(w_f4[:], 0.0)
        nc.vector.memset(w_g4[:], 0.0)
        for g in range(4):
            nc.sync.dma_start(w_f4[:, 32 * g:32 * g + C8], kf_d[:, :])
            nc.sync.dma_start(w_g4[:, 32 * g:32 * g + C8], kg_d[:, :])
        nc.sync.dma_start(w_h[:], kh_d[:, :])
        nc.sync.dma_start(w_o[:], ko_d[:, :])
        w_f4b = cpool.tile([C, P], BF16)
        w_g4b = cpool.tile([C, P], BF16)
        w_hb = cpool.tile([C, C], BF16)
        nc.vector.tensor_copy(w_f4b[:], w_f4[:])
        nc.vector.tensor_copy(w_g4b[:], w_g4[:])
        nc.vector.tensor_copy(w_hb[:], w_h[:])

        # ---- persistent activations ----------------------------------------
        xin = ppool.tile([P, NT, C], FP32)      # x, natural [n, c] layout
        f4 = ppool.tile([P, N], BF16)           # pooled fT, replicated 4x
        g4 = ppool.tile([P, N], BF16)           # gT, replicated 4x
        h_nt = ppool.tile([P, NT, C], BF16)     # pooled h in [n, c] layout
        o_rT = ppool.tile([C, N], FP32)         # reshaped-O transposed
        o_sb = ppool.tile([P, 2 * N // 4], FP32)  # O copied out of PSUM

        for t in range(NT):
            nc.sync.dma_start(xin[:, t, :], x_d[P * t:P * t + P, :])

        # ---- prepass: xT, convs, pooling, h transposes ---------------------
        with (
            tc.tile_pool(name="pre", bufs=1) as pre,
            tc.tile_pool(name="pre_ps", bufs=4, space="PSUM") as pre_ps,
        ):
            xT = pre.tile([C, N], BF16)
            hT = pre.tile([C, N], BF16)
            tmp = pre.tile([P, N], BF16)
            xb = pre.tile([P, NT, C], BF16)
            nc.vector.tensor_copy(xb[:], xin[:])

            # xT[c, n] via PE transposes (4 tiles per PSUM bank)
            for gidx in range(NT // 4):
                tp = pre_ps.tile([C, 4 * P], BF16, tag="ppb")
                for j in range(4):
                    t = 4 * gidx + j
                    nc.tensor.transpose(tp[:, P * j:P * j + P], xb[:, t, :],
                                        identb[:])
                nc.vector.tensor_copy(xT[:, 4 * P * gidx:4 * P * (gidx + 1)],
                                      tp[:])

            # 1x1 convs as matmuls from xT
            for blk in range(8):
                rhs = xT[:, 512 * blk:512 * blk + 512]
                psf = pre_ps.tile([P, 512], FP32, tag="pp")
                nc.tensor.matmul(psf[:], w_f4b[:], rhs, start=True, stop=True)
                nc.vector.tensor_copy(f4[:, 512 * blk:512 * blk + 512], psf[:])
                psg = pre_ps.tile([P, 512], FP32, tag="pp")
                nc.tensor.matmul(psg[:], w_g4b[:], rhs, start=True, stop=True)
                nc.vector.tensor_copy(g4[:, 512 * blk:512 * blk + 512], psg[:])
                psh = pre_ps.tile([C, 512], FP32, tag="pp")
                nc.tensor.matmul(psh[:], w_hb[:], rhs, start=True, stop=True)
                nc.vector.tensor_copy(hT[:, 512 * blk:512 * blk + 512], psh[:])

            # maxpool f (all 128 partitions: the 4 replicas pool identically)
            _pool2x2_s1_same(nc, f4[:], tmp[:], P)
            # maxpool h (64 partitions)
            _pool2x2_s1_same(nc, hT[:], tmp[0:C, :], C)

            # h_nt[n, c] via PE transposes (8 tiles per PSUM bank)
            for gidx in range(NT // 8):
                tp = pre_ps.tile([P, 8 * C], BF16, tag="ppb")
                for j in range(8):
                    t = 8 * gidx + j
                    nc.tensor.transpose(tp[:, C * j:C * j + C],
                                        hT[:, P * t:P * t + P],
                                        identb[0:C, 0:C])
                nc.vector.tensor_copy(
                    h_nt[:, 8 * gidx:8 * (gidx + 1), :].rearrange(
                        "p t c -> p (t c)"),
                    tp[:])

        # ---- main attention loop -------------------------------------------
        with (
            tc.tile_pool(name="o_ps", bufs=1, space="PSUM") as o_ps_pool,
            tc.tile_pool(name="e_ps", bufs=2, space="PSUM") as e_ps_pool,
            tc.tile_pool(name="p_sb", bufs=2) as p_pool,
            tc.tile_pool(name="s_sb", bufs=2) as s_pool,
        ):
            o_ps = o_ps_pool.tile([P, 2048], FP32)

            for t in range(NT):
                p_t = p_pool.tile([P, N], BF16, tag="p")
                s_parts = s_pool.tile([P, 4], FP32, tag="sp")
                for cch in range(4):
                    e_ps = e_ps_pool.tile([P, 1024], FP32, tag="e")
                    for j in range(2):
                        b = 2 * cch + j
                        grp = b % 4
                        nc.tensor.matmul(
                            e_ps[:, 512 * j:512 * j + 512],
                            f4[32 * grp:32 * grp + C8, P * t:P * t + P],
                            g4[32 * grp:32 * grp + C8, 512 * b:512 * b + 512],
                            start=True, stop=True,
                            tile_position=(32 * grp, 0))
                    nc.scalar.activation(
                        p_t[:, 1024 * cch:1024 * cch + 1024], e_ps[:],
                        AF.Exp, accum_out=s_parts[:, cch:cch + 1])

                s_sum = s_pool.tile([P, 1], tag="ss", dtype=FP32)
                r_t = s_pool.tile([P, 1], tag="rr", dtype=FP32)
                nc.vector.reduce_sum(s_sum[:], s_parts[:],
                                     axis=mybir.AxisListType.X)
                nc.vector.reciprocal(r_t[:], s_sum[:])
                hp_t = s_pool.tile([P, C], tag="hp", dtype=BF16)
                nc.vector.tensor_scalar_mul(hp_t[:], h_nt[:, t, :], r_t[:])

                for b in range(8):
                    obase = 64 * (b // 4)
                    col = 512 * (b % 4)
                    nc.tensor.matmul(
                        o_ps[obase:obase + 64, col:col + 512],
                        hp_t[:],
                        p_t[:, 512 * b:512 * b + 512],
                        start=(t == 0), stop=(t == NT - 1))

            nc.vector.tensor_copy(o_sb[:], o_ps[:])

        # ---- epilogue: reshape transposes, final conv, residual, store -----
        with (
            tc.tile_pool(name="ep_ps", bufs=2, space="PSUM") as ep_ps,
            tc.tile_pool(name="y_sb", bufs=2) as y_pool,
        ):
            # o_rT[k, 64a+w] = O[a, 64w + k];  O[a, m]: partitions 0:64 hold
            # m<2048, partitions 64:128 hold m>=2048 (both at col m%2048).
            o_rT3 = o_rT.rearrange("k (a w) -> k a w", w=W)
            for gidx in range(8):
                tp = ep_ps.tile([C, 8 * C], FP32, tag="ort")
                for j in range(8):
                    w_ = 8 * gidx + j
                    if w_ < 32:
                        src = o_sb[0:64, 64 * w_:64 * w_ + 64]
                        idn = ident[0:64, 0:64]
                    else:
                        src = o_sb[64:128, 64 * w_ - 2048:64 * w_ - 2048 + 64]
                        idn = ident[64:128, 64:128]
                    nc.tensor.transpose(tp[:, C * j:C * j + C], src, idn)
                nc.vector.tensor_copy(
                    o_rT3[:, :, 8 * gidx:8 * (gidx + 1)],
                    tp.rearrange("k (j a) -> k a j", a=C))

            xin_f = xin.rearrange("p t c -> p (t c)")
            for gidx in range(4):
                fps = ep_ps.tile([P, 512], FP32, tag="fin")
                for j in range(8):
                    t = 8 * gidx + j
                    nc.tensor.matmul(fps[:, C * j:C * j + C],
                                     o_rT[:, P * t:P * t + P], w_o[:],
                                     start=True, stop=True)
                y_sb = y_pool.tile([P, 512], FP32, tag="y")
                nc.vector.tensor_add(y_sb[:], fps[:],
                                     xin_f[:, 512 * gidx:512 * gidx + 512])
                for j in range(8):
                    t = 8 * gidx + j
                    nc.sync.dma_start(y_d[P * t:P * t + P, :],
                                      y_sb[:, C * j:C * j + C])


def _get_nc():
    global _CACHED_NC
    if _CACHED_NC is None:
        _CACHED_NC = _build_nc()
    return _CACHED_NC


def kernel(**inputs):
    global _LAST_RESULT
    x = np.ascontiguousarray(np.asarray(inputs["inputs"], dtype=np.float32))
    kf = np.ascontiguousarray(
        np.asarray(inputs["kernel_f"], dtype=np.float32).reshape(C, C8))
    kg = np.ascontiguousarray(
        np.asarray(inputs["kernel_g"], dtype=np.float32).reshape(C, C8))
    kh = np.ascontiguousarray(
        np.asarray(inputs["kernel_h"], dtype=np.float32).reshape(C, C))
    gamma = float(np.asarray(inputs["gamma"]).reshape(()))
    ko = np.ascontiguousarray(
        np.asarray(inputs["kernel_o"], dtype=np.float32).reshape(C, C) * gamma)

    nc = _get_nc()
    in_maps = []
    for b in range(NCORES):
        in_maps.append({
            "x": np.ascontiguousarray(x[b].reshape(N, C)),
            "kf": kf, "kg": kg, "kh": kh, "ko": ko,
        })
    res = run_bass_kernel_spmd(nc, in_maps, core_ids=list(range(NCORES)),
                               trace=_TRACE)
    _LAST_RESULT = res
    out = np.stack(
        [res.results[b]["y"].reshape(H, W, C) for b in range(NCORES)], axis=0)
    return out


# revision 6
# speedup vs baseline: 2.0966x; 1.0269x over previous
"""Trainium2 Bass kernel for nn_ConvSelfAttention (B=8, H=W=64, C=64, C8=8).

Sharding: data-parallel over batch; core b computes batch item b entirely
on-chip (one-pass softmax attention, no (N,N) matrix ever hits HBM).

Per-core pipeline:
  x [4096,64] -> PE-transpose -> xT [64,4096]
  convs as matmuls from xT:  f4/g4 [128,4096] (f,g replicated at partition
  bases 0/32/64/96 so K=8 score matmuls can use 4x row-tiling), hT [64,4096]
  maxpool(2x2,s1,SAME) via shifted tensor_max on DVE (f4 and hT)
  hT -> PE-transpose -> h_nt [n,c] layout
  main loop over 32 query tiles (128 rows each):
    e = fT.T @ gT   (PSUM, 4x row-tiled)
    p = exp(e)      (ScalarE, fused row-sum accum_out -> S)
    h' = h * (1/S)  (DVE)
    O[c,m] += h'.T @ p  (PSUM accumulate across all 32 tiles, 2x col-tiled:
                         m<2048 -> psum partitions 0:64, else 64:128)
  epilogue: reshape (C,N)->(H,W,C) via 64 PE 64x64 block transposes,
  conv with (gamma*Ko), residual add with x, DMA out.
"""

import numpy as np

import concourse.bass as bass
import concourse.mybir as mybir
import concourse.tile as tile
from concourse import bacc
from concourse.bass_utils import run_bass_kernel_spmd
from concourse.masks import make_identity

FP32 = mybir.dt.float32
BF16 = mybir.dt.bfloat16
AF = mybir.ActivationFunctionType

B, H, W, C = 8, 64, 64, 64
C8 = 8
N = H * W          # 4096
P = 128
NT = N // P        # 32 query tiles
NCORES = 8

_TRACE = False
_LAST_RESULT = None
_CACHED_NC = None


def _build_nc():
    nc = bacc.Bacc("TRN2", target_bir_lowering=False, debug=False,
                   num_devices=NCORES)

    x_d = nc.dram_tensor("x", [N, C], FP32, kind="ExternalInput")
    kf_d = nc.dram_tensor("kf", [C, C8], FP32, kind="ExternalInput")
    kg_d = nc.dram_tensor("kg", [C, C8], FP32, kind="ExternalInput")
    kh_d = nc.dram_tensor("kh", [C, C], FP32, kind="ExternalInput")
    ko_d = nc.dram_tensor("ko", [C, C], FP32, kind="ExternalInput")
    y_d = nc.dram_tensor("y", [N, C], FP32, kind="ExternalOutput")

    with tile.TileContext(nc) as tc:
        _kernel_body(nc, tc, x_d, kf_d, kg_d, kh_d, ko_d, y_d)
    nc.compile()
    return nc


def _pool2x2_s1_same(nc, src, tmp, pp):
    """maxpool window 2x2 stride 1 SAME over trailing (H, W) of [pp, 64, 64].

    src/tmp are flat [pp, 4096] APs viewed as [pp, i, j]; result written back
    into src (in place, via tmp)."""
    s3 = src.rearrange("p (i j) -> p i j", j=W)
    t3 = tmp.rearrange("p (i j) -> p i j", j=W)
    # horizontal: tmp[i, j] = max(src[i, j], src[i, j+1]) (last col pass-through)
    nc.vector.tensor_max(t3[:, :, 0:W - 1], s3[:, :, 0:W - 1], s3[:, :, 1:W])
    nc.vector.tensor_copy(t3[:, :, W - 1:W], s3[:, :, W - 1:W])
    # vertical: src[i, j] = max(tmp[i, j], tmp[i+1, j])
    nc.vector.tensor_max(s3[:, 0:H - 1, :], t3[:, 0:H - 1, :], t3[:, 1:H, :])
    nc.vector.tensor_copy(s3[:, H - 1:H, :], t3[:, H - 1:H, :])


def _kernel_body(nc, tc, x_d, kf_d, kg_d, kh_d, ko_d, y_d):
    with (
        tc.tile_pool(name="const", bufs=1) as cpool,
        tc.tile_pool(name="persist", bufs=1) as ppool,
    ):
        # ---- constants / weights -------------------------------------------
        ident = cpool.tile([P, P], FP32)
        make_identity(nc, ident[:])
        identb = cpool.tile([P, P], BF16)
        make_identity(nc, identb[:])

        w_f4 = cpool.tile([C, P], FP32)     # Kf replicated at cols 32g..32g+8
        w_g4 = cpool.tile([C, P], FP32)
        w_h = cpool.tile([C, C], FP32)
        w_o = cpool.tile([C, C], FP32)
        nc.vector.memset(w_f4[:], 0.0)
        nc.vector.memset(w_g4[:], 0.0)
        for g in range(4):
            nc.sync.dma_start(w_f4[:, 32 * g:32 * g + C8], kf_d[:, :])
            nc.sync.dma_start(w_g4[:, 32 * g:32 * g + C8], kg_d[:, :])
        nc.sync.dma_start(w_h[:], kh_d[:, :])
        nc.sync.dma_start(w_o[:], ko_d[:, :])
        w_ob = cpool.tile([C, C], BF16)
        nc.vector.tensor_copy(w_ob[:], w_o[:])
        w_f4b = cpool.tile([C, P], BF16)
        w_g4b = cpool.tile([C, P], BF16)
        w_hb = cpool.tile([C, C], BF16)
        nc.vector.tensor_copy(w_f4b[:], w_f4[:])
        nc.vector.tensor_copy(w_g4b[:], w_g4[:])
        nc.vector.tensor_copy(w_hb[:], w_h[:])

        # ---- persistent activations ----------------------------------------
        xin = ppool.tile([P, NT, C], FP32)      # x, natural [n, c] layout
        f4 = ppool.tile([P, N], BF16)           # pooled fT, replicated 4x
        g4 = ppool.tile([P, N], BF16)           # gT, replicated 4x
        h_nt = ppool.tile([P, NT, C], BF16)     # pooled h in [n, c] layout
        o_rT = ppool.tile([C, N], BF16)         # reshaped-O transposed
        o_sb = ppool.tile([P, 2 * N // 4], BF16)  # O copied out of PSUM

        x_pv = x_d.rearrange("(t p) c -> p t c", p=P)
        for g in range(8):
            nc.sync.dma_start(xin[:, 4 * g:4 * g + 4, :],
                              x_pv[:, 4 * g:4 * g + 4, :])

        # ---- prepass: xT, convs, pooling, h transposes ---------------------
        with (
            tc.tile_pool(name="pre", bufs=1) as pre,
            tc.tile_pool(name="pre_ps", bufs=4, space="PSUM") as pre_ps,
        ):
            xT = pre.tile([C, N], BF16)
            hT = pre.tile([C, N], BF16)
            tmp = pre.tile([P, N], BF16)
            xb = pre.tile([P, NT, C], BF16)

            # xT[c, n] via PE transposes (4 tiles per PSUM bank)
            for gidx in range(NT // 4):
                nc.vector.tensor_copy(xb[:, 4 * gidx:4 * gidx + 4, :],
                                      xin[:, 4 * gidx:4 * gidx + 4, :])
                tp = pre_ps.tile([C, 4 * P], BF16, tag="ppb")
                for j in range(4):
                    t = 4 * gidx + j
                    nc.tensor.transpose(tp[:, P * j:P * j + P], xb[:, t, :],
                                        identb[:])
                nc.vector.tensor_copy(xT[:, 4 * P * gidx:4 * P * (gidx + 1)],
                                      tp[:])

            # 1x1 convs as matmuls from xT
            for blk in range(8):
                rhs = xT[:, 512 * blk:512 * blk + 512]
                psf = pre_ps.tile([P, 512], FP32, tag="pp")
                nc.tensor.matmul(psf[:], w_f4b[:], rhs, start=True, stop=True)
                nc.vector.tensor_copy(f4[:, 512 * blk:512 * blk + 512], psf[:])
                psg = pre_ps.tile([P, 512], FP32, tag="pp")
                nc.tensor.matmul(psg[:], w_g4b[:], rhs, start=True, stop=True)
                nc.vector.tensor_copy(g4[:, 512 * blk:512 * blk + 512], psg[:])
                psh = pre_ps.tile([C, 512], FP32, tag="pp")
                nc.tensor.matmul(psh[:], w_hb[:], rhs, start=True, stop=True)
                nc.vector.tensor_copy(hT[:, 512 * blk:512 * blk + 512], psh[:])

            # maxpool h first: the PE h-transposes depend on it, and they
            # overlap with the f pool on DVE
            _pool2x2_s1_same(nc, hT[:], tmp[0:C, :], C)
            # maxpool f (all 128 partitions: the 4 replicas pool identically)
            _pool2x2_s1_same(nc, f4[:], tmp[:], P)

            # h_nt[n, c] via PE transposes (8 tiles per PSUM bank)
            for gidx in range(NT // 8):
                tp = pre_ps.tile([P, 8 * C], BF16, tag="ppb")
                for j in range(8):
                    t = 8 * gidx + j
                    nc.tensor.transpose(tp[:, C * j:C * j + C],
                                        hT[:, P * t:P * t + P],
                                        identb[0:C, 0:C])
                nc.vector.tensor_copy(
                    h_nt[:, 8 * gidx:8 * (gidx + 1), :].rearrange(
                        "p t c -> p (t c)"),
                    tp[:])

        # ---- main attention loop -------------------------------------------
        with (
            tc.tile_pool(name="o_ps", bufs=1, space="PSUM") as o_ps_pool,
            tc.tile_pool(name="e_ps", bufs=2, space="PSUM") as e_ps_pool,
            tc.tile_pool(name="p_sb", bufs=2) as p_pool,
            tc.tile_pool(name="s_sb", bufs=2) as s_pool,
        ):
            o_ps = o_ps_pool.tile([P, 2048], FP32)

            # Software-pipelined by one tile: the PE issues tile t+1's score
            # matmuls before tile t's O matmuls so it never sits behind the
            # exp->sum->scale chain of the current tile.
            prev = None  # (p_t, hp_t) of the previous tile

            def emit_scores(t):
                p_t = p_pool.tile([P, N], BF16, tag="p", name=f"p_{t}")
                s_parts = s_pool.tile([P, 4], FP32, tag="sp", name=f"sp_{t}")
                for cch in range(4):
                    e_ps = e_ps_pool.tile([P, 1024], FP32, tag="e",
                                          name=f"e_{t}_{cch}")
                    for j in range(2):
                        b = 2 * cch + j
                        grp = b % 4
                        nc.tensor.matmul(
                            e_ps[:, 512 * j:512 * j + 512],
                            f4[32 * grp:32 * grp + C8, P * t:P * t + P],
                            g4[32 * grp:32 * grp + C8, 512 * b:512 * b + 512],
                            start=True, stop=True,
                            tile_position=(32 * grp, 0))
                    nc.scalar.activation(
                        p_t[:, 1024 * cch:1024 * cch + 1024], e_ps[:],
                        AF.Exp, accum_out=s_parts[:, cch:cch + 1])
                s_sum = s_pool.tile([P, 1], tag="ss", dtype=FP32,
                                    name=f"ss_{t}")
                r_t = s_pool.tile([P, 1], tag="rr", dtype=FP32,
                                  name=f"r_{t}")
                nc.vector.reduce_sum(s_sum[:], s_parts[:],
                                     axis=mybir.AxisListType.X)
                nc.vector.reciprocal(r_t[:], s_sum[:])
                hp_t = s_pool.tile([P, C], tag="hp", dtype=BF16,
                                   name=f"hp_{t}")
                nc.vector.tensor_scalar_mul(hp_t[:], h_nt[:, t, :], r_t[:])
                return p_t, hp_t

            def emit_ov(t, p_t, hp_t):
                for b in range(8):
                    obase = 64 * (b // 4)
                    col = 512 * (b % 4)
                    nc.tensor.matmul(
                        o_ps[obase:obase + 64, col:col + 512],
                        hp_t[:],
                        p_t[:, 512 * b:512 * b + 512],
                        start=(t == 0), stop=(t == NT - 1))

            for t in range(NT):
                cur = emit_scores(t)
                if prev is not None:
                    emit_ov(t - 1, *prev)
                prev = cur
            emit_ov(NT - 1, *prev)

            nc.vector.tensor_copy(o_sb[:, 0:1024], o_ps[:, 0:1024])
            nc.vector.tensor_copy(o_sb[:, 1024:2048], o_ps[:, 1024:2048])

        # ---- epilogue: reshape transposes, final conv, residual, store -----
        with (
            tc.tile_pool(name="ep_ps", bufs=2, space="PSUM") as ep_ps,
            tc.tile_pool(name="y_sb", bufs=2) as y_pool,
        ):
            # o_rT[k, 64a+w] = O[a, 64w + k];  O[a, m]: partitions 0:64 hold
            # m<2048, partitions 64:128 hold m>=2048 (both at col m%2048).
            o_rT3 = o_rT.rearrange("k (a w) -> k a w", w=W)
            for gidx in range(8):
                tp = ep_ps.tile([C, 8 * C], BF16, tag="ort")
                for j in range(8):
                    w_ = 8 * gidx + j
                    if w_ < 32:
                        src = o_sb[0:64, 64 * w_:64 * w_ + 64]
                        idn = identb[0:64, 0:64]
                    else:
                        src = o_sb[64:128, 64 * w_ - 2048:64 * w_ - 2048 + 64]
                        idn = identb[64:128, 64:128]
                    nc.tensor.transpose(tp[:, C * j:C * j + C], src, idn)
                nc.vector.tensor_copy(
                    o_rT3[:, :, 8 * gidx:8 * (gidx + 1)],
                    tp.rearrange("k (j a) -> k a j", a=C))

            xin_f = xin.rearrange("p t c -> p (t c)")
            y_pv = y_d.rearrange("(t p) c -> p t c", p=P)
            for gidx in range(4):
                fps = ep_ps.tile([P, 512], FP32, tag="fin")
                for j in range(8):
                    t = 8 * gidx + j
                    nc.tensor.matmul(fps[:, C * j:C * j + C],
                                     o_rT[:, P * t:P * t + P], w_ob[:],
                                     start=True, stop=True)
                y_sb = y_pool.tile([P, 512], FP32, tag="y")
                nc.vector.tensor_add(y_sb[:], fps[:],
                                     xin_f[:, 512 * gidx:512 * gidx + 512])
                nc.sync.dma_start(
                    y_pv[:, 8 * gidx:8 * gidx + 8, :],
                    y_sb.rearrange("p (t c) -> p t c", c=C))


def _get_nc():
    global _CACHED_NC
    if _CACHED_NC is None:
        _CACHED_NC = _build_nc()
    return _CACHED_NC


def kernel(**inputs):
    global _LAST_RESULT
    x = np.ascontiguousarray(np.asarray(inputs["inputs"], dtype=np.float32))
    kf = np.ascontiguousarray(
        np.asarray(inputs["kernel_f"], dtype=np.float32).reshape(C, C8))
    kg = np.ascontiguousarray(
        np.asarray(inputs["kernel_g"], dtype=np.float32).reshape(C, C8))
    kh = np.ascontiguousarray(
        np.asarray(inputs["kernel_h"], dtype=np.float32).reshape(C, C))
    gamma = float(np.asarray(inputs["gamma"]).reshape(()))
    ko = np.ascontiguousarray(
        np.asarray(inputs["kernel_o"], dtype=np.float32).reshape(C, C) * gamma)

    nc = _get_nc()
    in_maps = []
    for b in range(NCORES):
        in_maps.append({
            "x": np.ascontiguousarray(x[b].reshape(N, C)),
            "kf": kf, "kg": kg, "kh": kh, "ko": ko,
        })
    res = run_bass_kernel_spmd(nc, in_maps, core_ids=list(range(NCORES)),
                               trace=_TRACE)
    _LAST_RESULT = res
    out = np.stack(
        [res.results[b]["y"].reshape(H, W, C) for b in range(NCORES)], axis=0)
    return out


# revision 7
# speedup vs baseline: 2.5073x; 1.1959x over previous
"""Trainium2 Bass kernel for nn_ConvSelfAttention (B=8, H=W=64, C=64, C8=8).

Sharding: data-parallel over batch; core b computes batch item b entirely
on-chip (one-pass softmax attention, no (N,N) matrix ever hits HBM).

Per-core pipeline:
  x [4096,64] -> PE-transpose -> xT [64,4096]
  convs as matmuls from xT:  f4/g4 [128,4096] (f,g replicated at partition
  bases 0/32/64/96 so K=8 score matmuls can use 4x row-tiling), hT [64,4096]
  maxpool(2x2,s1,SAME) via shifted tensor_max on DVE (f4 and hT)
  hT -> PE-transpose -> h_nt [n,c] layout
  main loop over 32 query tiles (128 rows each):
    e = fT.T @ gT   (PSUM, 4x row-tiled)
    p = exp(e)      (ScalarE, fused row-sum accum_out -> S)
    h' = h * (1/S)  (DVE)
    O[c,m] += h'.T @ p  (PSUM accumulate across all 32 tiles, 2x col-tiled:
                         m<2048 -> psum partitions 0:64, else 64:128)
  epilogue: reshape (C,N)->(H,W,C) via 64 PE 64x64 block transposes,
  conv with (gamma*Ko), residual add with x, DMA out.
"""

import numpy as np

import concourse.bass as bass
import concourse.mybir as mybir
import concourse.tile as tile
from concourse import bacc
from concourse.bass_utils import run_bass_kernel_spmd
from concourse.masks import make_identity

FP32 = mybir.dt.float32
BF16 = mybir.dt.bfloat16
AF = mybir.ActivationFunctionType

B, H, W, C = 8, 64, 64, 64
C8 = 8
N = H * W          # 4096
P = 128
NT = N // P        # 32 query tiles
NCORES = 8

_TRACE = False
_LAST_RESULT = None
_CACHED_NC = None


def _build_nc():
    nc = bacc.Bacc("TRN2", target_bir_lowering=False, debug=False,
                   num_devices=NCORES)

    x_d = nc.dram_tensor("x", [N, C], FP32, kind="ExternalInput")
    kf_d = nc.dram_tensor("kf", [C, C8], FP32, kind="ExternalInput")
    kg_d = nc.dram_tensor("kg", [C, C8], FP32, kind="ExternalInput")
    kh_d = nc.dram_tensor("kh", [C, C], FP32, kind="ExternalInput")
    ko_d = nc.dram_tensor("ko", [C, C], FP32, kind="ExternalInput")
    y_d = nc.dram_tensor("y", [N, C], FP32, kind="ExternalOutput")

    with tile.TileContext(nc) as tc:
        _kernel_body(nc, tc, x_d, kf_d, kg_d, kh_d, ko_d, y_d)
    nc.compile()
    return nc


def _pool2x2_s1_same(nc, src, tmp, pp):
    """maxpool window 2x2 stride 1 SAME over trailing (H, W) of [pp, 64, 64].

    src/tmp are flat [pp, 4096] APs viewed as [pp, i, j]; result written back
    into src (in place, via tmp)."""
    s3 = src.rearrange("p (i j) -> p i j", j=W)
    t3 = tmp.rearrange("p (i j) -> p i j", j=W)
    # horizontal: tmp[i, j] = max(src[i, j], src[i, j+1]) (last col pass-through)
    nc.vector.tensor_max(t3[:, :, 0:W - 1], s3[:, :, 0:W - 1], s3[:, :, 1:W])
    nc.vector.tensor_copy(t3[:, :, W - 1:W], s3[:, :, W - 1:W])
    # vertical: src[i, j] = max(tmp[i, j], tmp[i+1, j])
    nc.vector.tensor_max(s3[:, 0:H - 1, :], t3[:, 0:H - 1, :], t3[:, 1:H, :])
    nc.vector.tensor_copy(s3[:, H - 1:H, :], t3[:, H - 1:H, :])


def _kernel_body(nc, tc, x_d, kf_d, kg_d, kh_d, ko_d, y_d):
    with (
        tc.tile_pool(name="const", bufs=1) as cpool,
        tc.tile_pool(name="persist", bufs=1) as ppool,
    ):
        # ---- constants / weights -------------------------------------------
        ident = cpool.tile([P, P], FP32)
        make_identity(nc, ident[:])
        identb = cpool.tile([P, P], BF16)
        make_identity(nc, identb[:])

        w_f4 = cpool.tile([C, P], FP32)     # Kf replicated at cols 32g..32g+8
        w_g4 = cpool.tile([C, P], FP32)
        w_h = cpool.tile([C, C], FP32)
        w_o = cpool.tile([C, C], FP32)
        nc.vector.memset(w_f4[:], 0.0)
        nc.vector.memset(w_g4[:], 0.0)
        for g in range(4):
            nc.sync.dma_start(w_f4[:, 32 * g:32 * g + C8], kf_d[:, :])
            nc.sync.dma_start(w_g4[:, 32 * g:32 * g + C8], kg_d[:, :])
        nc.sync.dma_start(w_h[:], kh_d[:, :])
        nc.sync.dma_start(w_o[:], ko_d[:, :])
        w_ob = cpool.tile([C, C], BF16)
        nc.vector.tensor_copy(w_ob[:], w_o[:])
        w_f4b = cpool.tile([C, P], BF16)
        w_g4b = cpool.tile([C, P], BF16)
        w_hb = cpool.tile([C, C], BF16)
        nc.vector.tensor_copy(w_f4b[:], w_f4[:])
        nc.vector.tensor_copy(w_g4b[:], w_g4[:])
        nc.vector.tensor_copy(w_hb[:], w_h[:])

        # ---- persistent activations ----------------------------------------
        xin = ppool.tile([P, NT, C], FP32)      # x, natural [n, c] layout
        f4 = ppool.tile([P, N], BF16)           # pooled fT, replicated 4x
        g4 = ppool.tile([P, N], BF16)           # gT, replicated 4x
        h_nt = ppool.tile([P, NT, C], BF16)     # pooled h in [n, c] layout
        o_rT = ppool.tile([C, N], BF16)         # reshaped-O transposed
        o_sb = ppool.tile([P, 2 * N // 4], BF16)  # O copied out of PSUM

        x_pv = x_d.rearrange("(t p) c -> p t c", p=P)
        for g in range(8):
            nc.sync.dma_start(xin[:, 4 * g:4 * g + 4, :],
                              x_pv[:, 4 * g:4 * g + 4, :])

        # ---- prepass: xT, convs, pooling, h transposes ---------------------
        with (
            tc.tile_pool(name="pre", bufs=1) as pre,
            tc.tile_pool(name="pre_ps", bufs=4, space="PSUM") as pre_ps,
        ):
            xT = pre.tile([C, N], BF16)
            hT = pre.tile([C, N], BF16)
            tmp = pre.tile([P, N], BF16)
            xb = pre.tile([P, NT, C], BF16)

            # xT[c, n] via PE transposes (4 tiles per PSUM bank)
            for gidx in range(NT // 4):
                nc.vector.tensor_copy(xb[:, 4 * gidx:4 * gidx + 4, :],
                                      xin[:, 4 * gidx:4 * gidx + 4, :])
                tp = pre_ps.tile([C, 4 * P], BF16, tag="ppb")
                for j in range(4):
                    t = 4 * gidx + j
                    nc.tensor.transpose(tp[:, P * j:P * j + P], xb[:, t, :],
                                        identb[:])
                nc.vector.tensor_copy(xT[:, 4 * P * gidx:4 * P * (gidx + 1)],
                                      tp[:])

            # 1x1 convs as matmuls from xT
            for blk in range(8):
                rhs = xT[:, 512 * blk:512 * blk + 512]
                psf = pre_ps.tile([P, 512], FP32, tag="pp")
                nc.tensor.matmul(psf[:], w_f4b[:], rhs, start=True, stop=True)
                nc.vector.tensor_copy(f4[:, 512 * blk:512 * blk + 512], psf[:])
                psg = pre_ps.tile([P, 512], FP32, tag="pp")
                nc.tensor.matmul(psg[:], w_g4b[:], rhs, start=True, stop=True)
                nc.vector.tensor_copy(g4[:, 512 * blk:512 * blk + 512], psg[:])
                psh = pre_ps.tile([C, 512], FP32, tag="pp")
                nc.tensor.matmul(psh[:], w_hb[:], rhs, start=True, stop=True)
                nc.vector.tensor_copy(hT[:, 512 * blk:512 * blk + 512], psh[:])

            # maxpool h first: the PE h-transposes depend on it, and they
            # overlap with the f pool on DVE
            _pool2x2_s1_same(nc, hT[:], tmp[0:C, :], C)
            # maxpool f (all 128 partitions: the 4 replicas pool identically)
            _pool2x2_s1_same(nc, f4[:], tmp[:], P)

            # h_nt[n, c] via PE transposes (8 tiles per PSUM bank)
            for gidx in range(NT // 8):
                tp = pre_ps.tile([P, 8 * C], BF16, tag="ppb")
                for j in range(8):
                    t = 8 * gidx + j
                    nc.tensor.transpose(tp[:, C * j:C * j + C],
                                        hT[:, P * t:P * t + P],
                                        identb[0:C, 0:C])
                nc.vector.tensor_copy(
                    h_nt[:, 8 * gidx:8 * (gidx + 1), :].rearrange(
                        "p t c -> p (t c)"),
                    tp[:])

        # ---- main attention loop -------------------------------------------
        with (
            tc.tile_pool(name="o_ps", bufs=1, space="PSUM") as o_ps_pool,
            tc.tile_pool(name="e_ps", bufs=2, space="PSUM") as e_ps_pool,
            tc.tile_pool(name="p_sb", bufs=2) as p_pool,
            tc.tile_pool(name="s_sb", bufs=2) as s_pool,
        ):
            o_ps = o_ps_pool.tile([P, 2048], FP32)

            # Software-pipelined by one tile: the PE issues tile t+1's score
            # matmuls before tile t's O matmuls so it never sits behind the
            # exp->sum->scale chain of the current tile.
            prev = None  # (p_t, hp_t) of the previous tile

            def emit_scores(t):
                p_t = p_pool.tile([P, N], BF16, tag="p", name=f"p_{t}")
                s_parts = s_pool.tile([P, 4], FP32, tag="sp", name=f"sp_{t}")
                for cch in range(4):
                    e_ps = e_ps_pool.tile([P, 1024], FP32, tag="e",
                                          name=f"e_{t}_{cch}")
                    for j in range(2):
                        b = 2 * cch + j
                        grp = b % 4
                        nc.tensor.matmul(
                            e_ps[:, 512 * j:512 * j + 512],
                            f4[32 * grp:32 * grp + C8, P * t:P * t + P],
                            g4[32 * grp:32 * grp + C8, 512 * b:512 * b + 512],
                            start=True, stop=True,
                            tile_position=(32 * grp, 0))
                    nc.scalar.activation(
                        p_t[:, 1024 * cch:1024 * cch + 1024], e_ps[:],
                        AF.Exp, accum_out=s_parts[:, cch:cch + 1])
                s_sum = s_pool.tile([P, 1], tag="ss", dtype=FP32,
                                    name=f"ss_{t}")
                r_t = s_pool.tile([P, 1], tag="rr", dtype=FP32,
                                  name=f"r_{t}")
                nc.vector.reduce_sum(s_sum[:], s_parts[:],
                                     axis=mybir.AxisListType.X)
                nc.vector.reciprocal(r_t[:], s_sum[:])
                hp_t = s_pool.tile([P, C], tag="hp", dtype=BF16,
                                   name=f"hp_{t}")
                nc.vector.tensor_scalar_mul(hp_t[:], h_nt[:, t, :], r_t[:])
                return p_t, hp_t

            def emit_ov(t, p_t, hp_t):
                # alternate col-groups (psum partitions 0:64 / 64:128) so
                # adjacent matmuls run concurrently in the two array halves
                for b in (0, 4, 1, 5, 2, 6, 3, 7):
                    obase = 64 * (b // 4)
                    col = 512 * (b % 4)
                    nc.tensor.matmul(
                        o_ps[obase:obase + 64, col:col + 512],
                        hp_t[:],
                        p_t[:, 512 * b:512 * b + 512],
                        start=(t == 0), stop=(t == NT - 1))

            for t in range(NT):
                cur = emit_scores(t)
                if prev is not None:
                    emit_ov(t - 1, *prev)
                prev = cur
            emit_ov(NT - 1, *prev)

            nc.vector.tensor_copy(o_sb[:, 0:1024], o_ps[:, 0:1024])
            nc.vector.tensor_copy(o_sb[:, 1024:2048], o_ps[:, 1024:2048])

        # ---- epilogue: reshape transposes, final conv, residual, store -----
        with (
            tc.tile_pool(name="ep_ps", bufs=2, space="PSUM") as ep_ps,
            tc.tile_pool(name="y_sb", bufs=2) as y_pool,
        ):
            # o_rT[k, 64a+w] = O[a, 64w + k];  O[a, m]: partitions 0:64 hold
            # m<2048, partitions 64:128 hold m>=2048 (both at col m%2048).
            o_rT3 = o_rT.rearrange("k (a w) -> k a w", w=W)
            for gidx in range(8):
                tp = ep_ps.tile([C, 8 * C], BF16, tag="ort")
                for j in range(8):
                    w_ = 8 * gidx + j
                    if w_ < 32:
                        src = o_sb[0:64, 64 * w_:64 * w_ + 64]
                        idn = identb[0:64, 0:64]
                    else:
                        src = o_sb[64:128, 64 * w_ - 2048:64 * w_ - 2048 + 64]
                        idn = identb[64:128, 64:128]
                    nc.tensor.transpose(tp[:, C * j:C * j + C], src, idn)
                nc.vector.tensor_copy(
                    o_rT3[:, :, 8 * gidx:8 * (gidx + 1)],
                    tp.rearrange("k (j a) -> k a j", a=C))

            xin_f = xin.rearrange("p t c -> p (t c)")
            y_pv = y_d.rearrange("(t p) c -> p t c", p=P)
            for gidx in range(4):
                fps = ep_ps.tile([P, 512], FP32, tag="fin")
                for j in range(8):
                    t = 8 * gidx + j
                    nc.tensor.matmul(fps[:, C * j:C * j + C],
                                     o_rT[:, P * t:P * t + P], w_ob[:],
                                     start=True, stop=True)
                y_sb = y_pool.tile([P, 512], FP32, tag="y")
                nc.vector.tensor_add(y_sb[:], fps[:],
                                     xin_f[:, 512 * gidx:512 * gidx + 512])
                nc.sync.dma_start(
                    y_pv[:, 8 * gidx:8 * gidx + 8, :],
                    y_sb.rearrange("p (t c) -> p t c", c=C))


def _get_nc():
    global _CACHED_NC
    if _CACHED_NC is None:
        _CACHED_NC = _build_nc()
    return _CACHED_NC


def kernel(**inputs):
    global _LAST_RESULT
    x = np.ascontiguousarray(np.asarray(inputs["inputs"], dtype=np.float32))
    kf = np.ascontiguousarray(
        np.asarray(inputs["kernel_f"], dtype=np.float32).reshape(C, C8))
    kg = np.ascontiguousarray(
        np.asarray(inputs["kernel_g"], dtype=np.float32).reshape(C, C8))
    kh = np.ascontiguousarray(
        np.asarray(inputs["kernel_h"], dtype=np.float32).reshape(C, C))
    gamma = float(np.asarray(inputs["gamma"]).reshape(()))
    ko = np.ascontiguousarray(
        np.asarray(inputs["kernel_o"], dtype=np.float32).reshape(C, C) * gamma)

    nc = _get_nc()
    in_maps = []
    for b in range(NCORES):
        in_maps.append({
            "x": np.ascontiguousarray(x[b].reshape(N, C)),
            "kf": kf, "kg": kg, "kh": kh, "ko": ko,
        })
    res = run_bass_kernel_spmd(nc, in_maps, core_ids=list(range(NCORES)),
                               trace=_TRACE)
    _LAST_RESULT = res
    out = np.stack(
        [res.results[b]["y"].reshape(H, W, C) for b in range(NCORES)], axis=0)
    return out
